# revision 9
# baseline (speedup 1.0000x reference)
"""Trainium2 Bass kernel for nn_Actor (GNN message passing, B=65536, N=49).

Strategy
--------
The graph is fixed per call (edge_index + all weights are tiny inputs), so on
the host we fold the mean-aggregation matrix A (49x49) and the three SAGEConv
layers into dense matrices (float64 fold, exact). sage3 has no activation so
it folds into fc1. The network becomes a per-row MLP:

    [49] -> M1 [49,294] -> relu -> M2 [294,294] -> relu
         -> M3 [294,512] -> relu -> W4 [512,512] -> relu
         -> W5 [512,3] -> tanh -> *action_scale + action_bias

(294 = 6 feats x 49 nodes, feature-major order p = k*49 + n.)

Device side (8 NeuronCores, pure data parallel over batch), per core:

* x is transposed on the HOST into [49, 8192] fp16 and packed two
  batch-tiles per 128 SBUF partitions (even tiles at rows 0-48, odd at
  64-112), so there are no on-device transposes, no identity matrix and
  no batch permutation: DMA lands x directly in matmul layout.
* All folded weights live in ONE [128, 4888] fp16 DRAM blob (narrow
  K-chunks duplicated at row offsets 0/64), so the full weight load is
  3 fat DMAs with 128 descriptors each instead of ~1300 thin ones.
* Every layer is a stationary-weight matmul streaming the batch
  (BT=512 moving columns); bias+relu fuse into the PSUM->SBUF eviction
  on ScalarE/VectorE (alternated).
* PE array tiling: narrow K-chunks (49, 38) alternate row offsets 0/64
  across batch sub-tiles (dodges the LDWEIGHTS serialization stall of
  repeated narrow loads), narrow M-chunks (38 cols) alternate column
  offsets 0/64 (column tiling streams ~2x), and L5 (512->3, padded to
  32 cols) runs 4 batch sub-tiles at column groups 0/32/64/96 (~2.5x).
* L5 accumulates all four sub-tiles in one PSUM bank; a single fused
  tanh ACT evicts it, and per-sub-tile DMAs store [3, 512] slices in
  natural batch order.

Measured on 8 axon TRN2 cores: ~150 us NEFF exec (clock-dependent:
chip DVFS varies 2.0-2.4 GHz run to run), rel err ~9e-4 vs fp32 ref.
"""

import os
import sys

for _p in ("/opt/trn_rl_repo", "/root/.axon_site/_ro/trn_rl_repo"):
    if os.path.isdir(_p) and _p not in sys.path:
        sys.path.append(_p)

import numpy as np

N = 49
B = 65536
N_CORES = 8
B_CORE = B // N_CORES          # 8192
BT = 512                       # batch tile (matmul free dim)
N_TILES = B_CORE // BT         # 16
SUB = 4                        # batch-tiles per super-tile
N_ST = N_TILES // SUB          # 4

# Layer dims: K -> M
DIMS = [(49, 294), (294, 294), (294, 512), (512, 512), (512, 3)]

# base patterns per sub-tile s (which 64-row half of the partitions)
P_PAR = (0, 64, 0, 64)         # parity pattern (x layout, L2-out narrow)
P_ALT = (0, 64, 64, 0)         # alternate (L1-out narrow)
P_Z = (0, 0, 0, 0)

# k-chunks per layer: (kc, input-base-pattern or None for full)
KCH = [
    [(49, P_PAR)],
    [(128, P_Z), (128, P_Z), (38, P_ALT)],
    [(128, P_Z), (128, P_Z), (38, P_PAR)],
    [(128, P_Z)] * 4,
    [(128, P_Z)] * 4,
]
# m-chunks per layer (layers 0..3): (ms, mc, output-base-pattern)
MCH = [
    [(0, 128, P_Z), (128, 128, P_Z), (256, 38, P_ALT)],
    [(0, 128, P_Z), (128, 128, P_Z), (256, 38, P_PAR)],
    [(0, 128, P_Z), (128, 128, P_Z), (256, 128, P_Z), (384, 128, P_Z)],
    [(0, 128, P_Z), (128, 128, P_Z), (256, 128, P_Z), (384, 128, P_Z)],
]

# weight blob column layout: (layer, ki) -> (col_off, kc, dup)
WOFF = {}
_off = 0
for _l, _ch in enumerate(KCH):
    _, _M = DIMS[_l]
    _Mpad = 32 if _l == 4 else _M
    for _ki, (_kc, _pat) in enumerate(_ch):
        WOFF[(_l, _ki)] = (_off, _kc, _kc <= 64)
        _off += _Mpad
F_TOTAL = _off                 # 4888


# ----------------------------------------------------------------- host fold

def fold_weights(inputs):
    f8 = np.float64
    ei = np.asarray(inputs['edge_index'])
    src, dst = ei[0].astype(np.int64), ei[1].astype(np.int64)
    C = np.zeros((N, N), f8)
    np.add.at(C, (dst, src), 1.0)
    cnt = C.sum(axis=1)
    A = C / np.clip(cnt, 1.0, None)[:, None]
    I = np.eye(N, dtype=f8)

    W1l = np.asarray(inputs['W1l'], f8); W1r = np.asarray(inputs['W1r'], f8)
    b1 = np.asarray(inputs['b1'], f8)
    W2l = np.asarray(inputs['W2l'], f8); W2r = np.asarray(inputs['W2r'], f8)
    b2 = np.asarray(inputs['b2'], f8)
    W3l = np.asarray(inputs['W3l'], f8); W3r = np.asarray(inputs['W3r'], f8)
    b3 = np.asarray(inputs['b3'], f8)
    fc1_w = np.asarray(inputs['fc1_w'], f8); fc1_b = np.asarray(inputs['fc1_b'], f8)

    M1 = np.zeros((N, 6 * N), f8)
    for k in range(6):
        M1[:, k * N:(k + 1) * N] = A.T * W1l[0, k] + I * W1r[0, k]
    B1 = np.repeat(b1, N)

    M2 = np.zeros((6 * N, 6 * N), f8)
    for k in range(6):
        for k2 in range(6):
            M2[k * N:(k + 1) * N, k2 * N:(k2 + 1) * N] = \
                A.T * W2l[k, k2] + I * W2r[k, k2]
    B2 = np.repeat(b2, N)

    F = fc1_w.reshape(N, 12, 512)
    T1 = np.einsum('nm,kf,nfc->kmc', A, W3l, F, optimize=True)
    T2 = np.einsum('kf,mfc->kmc', W3r, F, optimize=True)
    M3 = (T1 + T2).reshape(6 * N, 512)
    B3 = np.einsum('f,nfc->c', b3, F, optimize=True) + fc1_b

    return dict(
        w=[M1, M2, M3, np.asarray(inputs['fc2_w'], f8), np.asarray(inputs['mu_w'], f8)],
        b=[B1, B2, B3, np.asarray(inputs['fc2_b'], f8), np.asarray(inputs['mu_b'], f8)],
        asc=np.asarray(inputs['action_scale'], f8),
        abi=np.asarray(inputs['action_bias'], f8),
    )


def build_blob(fw):
    """Assemble the [128, F_TOTAL] fp16 weight blob (dup narrow chunks)."""
    WB = np.zeros((128, F_TOTAL), np.float16)
    for layer in range(5):
        w = fw['w'][layer].astype(np.float16)
        M = DIMS[layer][1]
        Mpad = 32 if layer == 4 else M
        ks = 0
        for ki, (kc, _pat) in enumerate(KCH[layer]):
            off, _, dup = WOFF[(layer, ki)]
            WB[0:kc, off:off + M] = w[ks:ks + kc, :]
            if dup:
                WB[64:64 + kc, off:off + M] = w[ks:ks + kc, :]
            ks += kc
    return WB


def build_x2(xc):
    """xc [B_CORE, N] fp32 -> [98, B_CORE] fp16: even batch-tiles in rows
    0-48, odd tiles in rows 49-97 (landing on SBUF partitions 64-112)."""
    xr = xc.astype(np.float16).reshape(N_TILES, BT, N)
    top = xr[0::2].transpose(2, 0, 1).reshape(N, (N_TILES // 2) * BT)
    bot = xr[1::2].transpose(2, 0, 1).reshape(N, (N_TILES // 2) * BT)
    return np.ascontiguousarray(np.concatenate([top, bot], axis=0))


# -------------------------------------------------------------- bass builder

def build_nc(use_bias, use_scale):
    import concourse.mybir as mybir
    import concourse.tile as tile
    from concourse import bacc

    f16 = mybir.dt.float16
    f32 = mybir.dt.float32
    Relu = mybir.ActivationFunctionType.Relu
    Tanh = mybir.ActivationFunctionType.Tanh

    nc = bacc.Bacc("TRN2", target_bir_lowering=False, debug=False,
                   num_devices=N_CORES)

    H = (N_TILES // 2) * BT         # 4096 cols per parity half
    x2_d = nc.declare_dram_parameter("x2", [2 * N, H], f16, isOutput=False)
    wb_d = nc.declare_dram_parameter("wb", [128, F_TOTAL], f16, isOutput=False)
    out_d = nc.declare_dram_parameter("out", [3, B_CORE], f32, isOutput=True)
    b_d = None
    if use_bias:
        b_d = [nc.declare_dram_parameter(f"bp{i}", [len(MCH[i]) if i < 4 else 1, 128],
                                         f32, isOutput=False) for i in range(5)]
    s_d = None
    if use_scale:
        s_d = [nc.declare_dram_parameter("ascp", [1, 128], f32, isOutput=False),
               nc.declare_dram_parameter("abip", [1, 128], f32, isOutput=False)]

    with tile.TileContext(nc) as tc:
        with (
            tc.tile_pool(name="const", bufs=1) as const,
            tc.tile_pool(name="acts", bufs=36) as acts,
            tc.tile_pool(name="outs", bufs=4) as outs,
            tc.tile_pool(name="psum", bufs=8, space="PSUM") as psum,
        ):
            X2 = const.tile([128, H], f16, tag="x2")
            WB = const.tile([128, F_TOTAL], f16, tag="wb")

            C0 = 2 * BT                 # first super-tile needs cols [0:1024)

            # DMA trigger order matters: each dma_start costs ~0.7us on the
            # Sync queue, so the first matmul's deps (w0, then first-super-
            # tile x) go first, then weights layer by layer, then the rest.
            o1 = WOFF[(1, 0)][0]        # 294
            o2 = WOFF[(2, 0)][0]        # 1176
            o3 = WOFF[(3, 0)][0]        # 2712
            nc.sync.dma_start(out=WB[:, 0:o1], in_=wb_d[:, 0:o1])
            nc.sync.dma_start(out=X2[0:N, 0:BT], in_=x2_d[0:N, 0:BT])
            nc.sync.dma_start(out=X2[64:64 + N, 0:BT], in_=x2_d[N:2 * N, 0:BT])
            nc.sync.dma_start(out=X2[0:N, BT:C0], in_=x2_d[0:N, BT:C0])
            nc.sync.dma_start(out=X2[64:64 + N, BT:C0], in_=x2_d[N:2 * N, BT:C0])
            nc.sync.dma_start(out=WB[:, o1:o2], in_=wb_d[:, o1:o2])
            nc.sync.dma_start(out=WB[:, o2:o3], in_=wb_d[:, o2:o3])
            nc.sync.dma_start(out=WB[:, o3:F_TOTAL], in_=wb_d[:, o3:F_TOTAL])
            nc.sync.dma_start(out=X2[0:N, C0:H], in_=x2_d[0:N, C0:H])
            nc.sync.dma_start(out=X2[64:64 + N, C0:H], in_=x2_d[N:2 * N, C0:H])

            b_sb = None
            if use_bias:
                b_sb = []
                for i in range(5):
                    nm = len(MCH[i]) if i < 4 else 1
                    t = const.tile([128, nm], f32, tag=f"b{i}")
                    nc.sync.dma_start(out=t, in_=b_d[i].rearrange("m p -> p m"))
                    b_sb.append(t)
            if use_scale:
                asc_sb = const.tile([128, 1], f32, tag="asc")
                abi_sb = const.tile([128, 1], f32, tag="abi")
                nc.sync.dma_start(out=asc_sb, in_=s_d[0].rearrange("m p -> p m"))
                nc.sync.dma_start(out=abi_sb, in_=s_d[1].rearrange("m p -> p m"))

            def w_ap(layer, ki, ms, mc, rb):
                off, kc, dup = WOFF[(layer, ki)]
                base = rb if dup else 0
                return WB[base:base + kc, off + ms:off + ms + mc]

            # eviction engine round-robin: ScalarE (fused act) / VectorE
            rr = [0]

            def evict(layer, mi, h_ap, ps_ap, force=None):
                eng = force
                if eng is None:
                    eng = "s" if rr[0] % 2 == 0 else "v"
                    rr[0] += 1
                if eng == "s":
                    bias = 0.0
                    if use_bias:
                        ob = h_ap.base_partition()
                        mc = h_ap.partition_size()
                        bias = b_sb[layer][ob:ob + mc, mi:mi + 1]
                    nc.scalar.activation(out=h_ap, in_=ps_ap, func=Relu,
                                         bias=bias)
                elif use_bias:
                    ob = h_ap.base_partition()
                    mc = h_ap.partition_size()
                    nc.vector.tensor_scalar(
                        out=h_ap, in0=ps_ap,
                        scalar1=b_sb[layer][ob:ob + mc, mi:mi + 1], scalar2=0.0,
                        op0=mybir.AluOpType.add, op1=mybir.AluOpType.max)
                else:
                    nc.vector.tensor_scalar_max(h_ap, ps_ap, 0.0)

            def s_interleaved(pat):
                zs = [s for s in range(SUB) if pat[s] == 0]
                os_ = [s for s in range(SUB) if pat[s] == 64]
                out = []
                for a, b in zip(zs, os_):
                    out += [a, b]
                return out or list(range(SUB))

            for st in range(N_ST):
                # layer inputs: cur[ki][s] = (tile, row_base, col_off)
                cur = [[(X2, P_PAR[s], (2 * st + s // 2) * BT)
                        for s in range(SUB)]]

                h4 = []   # L4 output chunks for L5
                for layer in range(4):
                    kch = KCH[layer]
                    narrow = [ki for ki, (kc, _p) in enumerate(kch) if kc <= 64]
                    fulls = [ki for ki, (kc, _p) in enumerate(kch) if kc > 64]
                    nxt = []
                    for mi, (ms, mc, opat) in enumerate(MCH[layer]):
                        # merge narrow-k runs across m-chunk pairs
                        order = (narrow + fulls) if mi % 2 == 1 else (fulls + narrow)
                        pss = [psum.tile([128, BT], f32, name="ps", tag="ps")
                               for _ in range(SUB)]
                        for idx, ki in enumerate(order):
                            kc, ipat = kch[ki]
                            pat = ipat if kc <= 64 else (
                                opat if mc <= 64 else P_Z)
                            sseq = (s_interleaved(pat)
                                    if (kc <= 64 or mc <= 64)
                                    else range(SUB))
                            for s in sseq:
                                t_in, ib, co = cur[ki][s]
                                ob = opat[s]
                                nc.tensor.matmul(
                                    pss[s][ob:ob + mc, :],
                                    w_ap(layer, ki, ms, mc, ib),
                                    t_in[ib:ib + kc, co:co + BT],
                                    start=(idx == 0),
                                    stop=(idx == len(order) - 1),
                                    tile_position=(ib, ob))
                        hs = []
                        for s in range(SUB):
                            ob = opat[s]
                            h = acts.tile([128, BT], f16, name="h", tag="h")
                            force = "v" if (layer == 3 and mi == 3) else None
                            evict(layer, mi, h[ob:ob + mc, :],
                                  pss[s][ob:ob + mc, :], force)
                            hs.append((h, ob, 0))
                        nxt.append(hs)
                    cur = nxt
                h4 = cur

                # L5: 512 -> 3 (padded to 32 cols), 4 sub-tiles at column
                # groups 0/32/64/96 of one PSUM bank.
                ps5 = psum.tile([128, BT], f32, name="ps5", tag="ps")
                for ki in range(4):
                    off5 = WOFF[(4, ki)][0]
                    for s in range(SUB):
                        h, ob, _ = h4[ki][s]
                        nc.tensor.matmul(
                            ps5[32 * s:32 * s + 32, :],
                            WB[0:128, off5:off5 + 32],
                            h[0:128, :],
                            start=(ki == 0), stop=(ki == 3),
                            tile_position=(0, 32 * s))
                t5 = outs.tile([128, BT], f32, tag="t5")
                bias5 = b_sb[4][:, 0:1] if use_bias else 0.0
                nc.scalar.activation(out=t5, in_=ps5, func=Tanh, bias=bias5)
                if use_scale:
                    t5s = outs.tile([128, BT], f32, tag="t5s")
                    nc.vector.tensor_scalar(
                        out=t5s, in0=t5, scalar1=asc_sb, scalar2=abi_sb,
                        op0=mybir.AluOpType.mult, op1=mybir.AluOpType.add)
                    t5 = t5s
                for s in range(SUB):
                    T = 4 * st + s
                    nc.sync.dma_start(
                        out=out_d[:, T * BT:(T + 1) * BT],
                        in_=t5[32 * s:32 * s + 3, :])

                if st == N_ST - 1:
                    # tail warm: keep the PE busy through the tanh/store/
                    # teardown tail so the HAM doesn't re-throttle the
                    # clock to K=4/8 for the last few microseconds.
                    ps_w = psum.tile([128, BT], f32, name="psw", tag="ps")
                    hw_in = h4[3][3][0]
                    for _j in range(14):
                        nc.tensor.matmul(
                            ps_w[0:128, :], WB[0:128, o3:o3 + 128],
                            hw_in[0:128, :], start=True, stop=True)

    nc.compile()
    return nc


_CACHE = {}


def _get_nc(use_bias, use_scale):
    key = (use_bias, use_scale)
    if key not in _CACHE:
        _CACHE[key] = build_nc(use_bias, use_scale)
    return _CACHE[key]


def _pad_bias(vec, layer):
    """Pad a bias vector into the [n_mchunks, 128] layout (dup narrow)."""
    if layer == 4:
        out = np.zeros((1, 128), np.float32)
        for s in range(4):
            out[0, 32 * s:32 * s + 3] = vec[0:3]
        return out
    mch = MCH[layer]
    out = np.zeros((len(mch), 128), np.float32)
    for mi, (ms, mc, _p) in enumerate(mch):
        out[mi, 0:mc] = vec[ms:ms + mc]
        if mc <= 64:
            out[mi, 64:64 + mc] = vec[ms:ms + mc]
    return out


def prepare(inputs):
    """Fold weights and build per-core input maps. Returns (nc, in_maps)."""
    fw = fold_weights(inputs)
    use_bias = any(np.any(bi != 0.0) for bi in fw['b'])
    use_scale = bool(np.any(fw['asc'] != 1.0) or np.any(fw['abi'] != 0.0))
    nc = _get_nc(use_bias, use_scale)

    WB = build_blob(fw)
    x = np.asarray(inputs['x'], np.float32).reshape(B, N)

    in_maps = []
    for c in range(N_CORES):
        m = {"x2": build_x2(x[c * B_CORE:(c + 1) * B_CORE]), "wb": WB}
        if use_bias:
            for i in range(5):
                m[f"bp{i}"] = _pad_bias(fw['b'][i].astype(np.float32), i)
        if use_scale:
            m["ascp"] = _pad_bias(fw['asc'].astype(np.float32), 4)[0:1]
            m["abip"] = _pad_bias(fw['abi'].astype(np.float32), 4)[0:1]
        in_maps.append(m)
    return nc, in_maps


def gather(results):
    """Unshard: per-core out is [3, B_CORE] in natural batch order."""
    parts = [np.asarray(results[c]["out"]).T for c in range(N_CORES)]
    return np.ascontiguousarray(np.concatenate(parts, axis=0)
                                .astype(np.float32))


def kernel(**inputs):
    from concourse.bass_utils import run_bass_kernel_spmd

    nc, in_maps = prepare(inputs)
    res = run_bass_kernel_spmd(nc, in_maps, core_ids=list(range(N_CORES)))
    return gather(res.results)


# revision 11
# speedup vs baseline: 1.0126x; 1.0126x over previous
"""Trainium2 Bass kernel for nn_Actor (GNN message passing, B=65536, N=49).

Strategy
--------
The graph is fixed per call (edge_index + all weights are tiny inputs), so on
the host we fold the mean-aggregation matrix A (49x49) and the three SAGEConv
layers into dense matrices (float64 fold, exact). sage3 has no activation so
it folds into fc1. The network becomes a per-row MLP:

    [49] -> M1 [49,294] -> relu -> M2 [294,294] -> relu
         -> M3 [294,512] -> relu -> W4 [512,512] -> relu
         -> W5 [512,3] -> tanh -> *action_scale + action_bias

(294 = 6 feats x 49 nodes, feature-major order p = k*49 + n.)

Device side (8 NeuronCores, pure data parallel over batch), per core:

* x is transposed on the HOST into [49, 8192] fp16 and packed two
  batch-tiles per 128 SBUF partitions (even tiles at rows 0-48, odd at
  64-112), so there are no on-device transposes, no identity matrix and
  no batch permutation: DMA lands x directly in matmul layout.
* All folded weights live in ONE [128, 4888] fp16 DRAM blob (narrow
  K-chunks duplicated at row offsets 0/64), so the full weight load is
  3 fat DMAs with 128 descriptors each instead of ~1300 thin ones.
* Every layer is a stationary-weight matmul streaming the batch
  (BT=512 moving columns); bias+relu fuse into the PSUM->SBUF eviction
  on ScalarE/VectorE (alternated).
* PE array tiling: narrow K-chunks (49, 38) alternate row offsets 0/64
  across batch sub-tiles (dodges the LDWEIGHTS serialization stall of
  repeated narrow loads), narrow M-chunks (38 cols) alternate column
  offsets 0/64 (column tiling streams ~2x), and L5 (512->3, padded to
  32 cols) runs 4 batch sub-tiles at column groups 0/32/64/96 (~2.5x).
* L5 accumulates all four sub-tiles in one PSUM bank; a single fused
  tanh ACT evicts it, and per-sub-tile DMAs store [3, 512] slices in
  natural batch order.

Measured on 8 axon TRN2 cores: ~156 us NEFF exec at 2.37 GHz (DVFS can
vary run to run; matmul stream is ~136.5 us, at the fp16 streaming floor
for this pass count), rel err 9.4e-4 vs the fp32 reference (gate 2e-2).
Baseline before this rework: 177 us.
"""

import os
import sys

for _p in ("/opt/trn_rl_repo", "/root/.axon_site/_ro/trn_rl_repo"):
    if os.path.isdir(_p) and _p not in sys.path:
        sys.path.append(_p)

import numpy as np

N = 49
B = 65536
N_CORES = 8
B_CORE = B // N_CORES          # 8192
BT = 512                       # batch tile (matmul free dim)
N_TILES = B_CORE // BT         # 16
SUB = 4                        # batch-tiles per super-tile
N_ST = N_TILES // SUB          # 4

# Layer dims: K -> M
DIMS = [(49, 294), (294, 294), (294, 512), (512, 512), (512, 3)]

# base patterns per sub-tile s (which 64-row half of the partitions)
P_PAR = (0, 64, 0, 64)         # parity pattern (x layout, L2-out narrow)
P_ALT = (0, 64, 64, 0)         # alternate (L1-out narrow)
P_Z = (0, 0, 0, 0)

# k-chunks per layer: (kc, input-base-pattern or None for full)
KCH = [
    [(49, P_PAR)],
    [(128, P_Z), (128, P_Z), (38, P_ALT)],
    [(128, P_Z), (128, P_Z), (38, P_PAR)],
    [(128, P_Z)] * 4,
    [(128, P_Z)] * 4,
]
# m-chunks per layer (layers 0..3): (ms, mc, output-base-pattern)
MCH = [
    [(0, 128, P_Z), (128, 128, P_Z), (256, 38, P_ALT)],
    [(0, 128, P_Z), (128, 128, P_Z), (256, 38, P_PAR)],
    [(0, 128, P_Z), (128, 128, P_Z), (256, 128, P_Z), (384, 128, P_Z)],
    [(0, 128, P_Z), (128, 128, P_Z), (256, 128, P_Z), (384, 128, P_Z)],
]

# weight blob column layout: (layer, ki) -> (col_off, kc, dup)
WOFF = {}
_off = 0
for _l, _ch in enumerate(KCH):
    _, _M = DIMS[_l]
    _Mpad = 32 if _l == 4 else _M
    for _ki, (_kc, _pat) in enumerate(_ch):
        WOFF[(_l, _ki)] = (_off, _kc, _kc <= 64)
        _off += _Mpad
F_TOTAL = _off                 # 4888


# ----------------------------------------------------------------- host fold

def fold_weights(inputs):
    f8 = np.float64
    ei = np.asarray(inputs['edge_index'])
    src, dst = ei[0].astype(np.int64), ei[1].astype(np.int64)
    C = np.zeros((N, N), f8)
    np.add.at(C, (dst, src), 1.0)
    cnt = C.sum(axis=1)
    A = C / np.clip(cnt, 1.0, None)[:, None]
    I = np.eye(N, dtype=f8)

    W1l = np.asarray(inputs['W1l'], f8); W1r = np.asarray(inputs['W1r'], f8)
    b1 = np.asarray(inputs['b1'], f8)
    W2l = np.asarray(inputs['W2l'], f8); W2r = np.asarray(inputs['W2r'], f8)
    b2 = np.asarray(inputs['b2'], f8)
    W3l = np.asarray(inputs['W3l'], f8); W3r = np.asarray(inputs['W3r'], f8)
    b3 = np.asarray(inputs['b3'], f8)
    fc1_w = np.asarray(inputs['fc1_w'], f8); fc1_b = np.asarray(inputs['fc1_b'], f8)

    M1 = np.zeros((N, 6 * N), f8)
    for k in range(6):
        M1[:, k * N:(k + 1) * N] = A.T * W1l[0, k] + I * W1r[0, k]
    B1 = np.repeat(b1, N)

    M2 = np.zeros((6 * N, 6 * N), f8)
    for k in range(6):
        for k2 in range(6):
            M2[k * N:(k + 1) * N, k2 * N:(k2 + 1) * N] = \
                A.T * W2l[k, k2] + I * W2r[k, k2]
    B2 = np.repeat(b2, N)

    F = fc1_w.reshape(N, 12, 512)
    T1 = np.einsum('nm,kf,nfc->kmc', A, W3l, F, optimize=True)
    T2 = np.einsum('kf,mfc->kmc', W3r, F, optimize=True)
    M3 = (T1 + T2).reshape(6 * N, 512)
    B3 = np.einsum('f,nfc->c', b3, F, optimize=True) + fc1_b

    return dict(
        w=[M1, M2, M3, np.asarray(inputs['fc2_w'], f8), np.asarray(inputs['mu_w'], f8)],
        b=[B1, B2, B3, np.asarray(inputs['fc2_b'], f8), np.asarray(inputs['mu_b'], f8)],
        asc=np.asarray(inputs['action_scale'], f8),
        abi=np.asarray(inputs['action_bias'], f8),
    )


def build_blob(fw):
    """Assemble the [128, F_TOTAL] fp16 weight blob (dup narrow chunks)."""
    WB = np.zeros((128, F_TOTAL), np.float16)
    for layer in range(5):
        w = fw['w'][layer].astype(np.float16)
        M = DIMS[layer][1]
        Mpad = 32 if layer == 4 else M
        ks = 0
        for ki, (kc, _pat) in enumerate(KCH[layer]):
            off, _, dup = WOFF[(layer, ki)]
            WB[0:kc, off:off + M] = w[ks:ks + kc, :]
            if dup:
                WB[64:64 + kc, off:off + M] = w[ks:ks + kc, :]
            ks += kc
    return WB


def build_x2(xc):
    """xc [B_CORE, N] fp32 -> [98, B_CORE] fp16: even batch-tiles in rows
    0-48, odd tiles in rows 49-97 (landing on SBUF partitions 64-112)."""
    xr = xc.astype(np.float16).reshape(N_TILES, BT, N)
    top = xr[0::2].transpose(2, 0, 1).reshape(N, (N_TILES // 2) * BT)
    bot = xr[1::2].transpose(2, 0, 1).reshape(N, (N_TILES // 2) * BT)
    return np.ascontiguousarray(np.concatenate([top, bot], axis=0))


# -------------------------------------------------------------- bass builder

def build_nc(use_bias, use_scale):
    import concourse.mybir as mybir
    import concourse.tile as tile
    from concourse import bacc

    f16 = mybir.dt.float16
    f32 = mybir.dt.float32
    Relu = mybir.ActivationFunctionType.Relu
    Tanh = mybir.ActivationFunctionType.Tanh

    nc = bacc.Bacc("TRN2", target_bir_lowering=False, debug=False,
                   num_devices=N_CORES)

    H = (N_TILES // 2) * BT         # 4096 cols per parity half
    x2_d = nc.declare_dram_parameter("x2", [2 * N, H], f16, isOutput=False)
    wb_d = nc.declare_dram_parameter("wb", [128, F_TOTAL], f16, isOutput=False)
    out_d = nc.declare_dram_parameter("out", [3, B_CORE], f32, isOutput=True)
    b_d = None
    if use_bias:
        b_d = [nc.declare_dram_parameter(f"bp{i}", [len(MCH[i]) if i < 4 else 1, 128],
                                         f32, isOutput=False) for i in range(5)]
    s_d = None
    if use_scale:
        s_d = [nc.declare_dram_parameter("ascp", [1, 128], f32, isOutput=False),
               nc.declare_dram_parameter("abip", [1, 128], f32, isOutput=False)]

    with tile.TileContext(nc) as tc:
        with (
            tc.tile_pool(name="const", bufs=1) as const,
            tc.tile_pool(name="acts", bufs=36) as acts,
            tc.tile_pool(name="outs", bufs=4) as outs,
            tc.tile_pool(name="psum", bufs=8, space="PSUM") as psum,
        ):
            X2 = const.tile([128, H], f16, tag="x2")
            WB = const.tile([128, F_TOTAL], f16, tag="wb")

            C0 = 2 * BT                 # first super-tile needs cols [0:1024)

            # DMA trigger order matters: each dma_start costs ~0.7us on the
            # Sync queue, so the first matmul's deps (w0, then first-super-
            # tile x) go first, then weights layer by layer, then the rest.
            o1 = WOFF[(1, 0)][0]        # 294
            o2 = WOFF[(2, 0)][0]        # 1176
            o3 = WOFF[(3, 0)][0]        # 2712
            nc.sync.dma_start(out=WB[:, 0:o1], in_=wb_d[:, 0:o1])
            nc.sync.dma_start(out=X2[0:N, 0:BT], in_=x2_d[0:N, 0:BT])
            nc.sync.dma_start(out=X2[64:64 + N, 0:BT], in_=x2_d[N:2 * N, 0:BT])
            nc.sync.dma_start(out=X2[0:N, BT:C0], in_=x2_d[0:N, BT:C0])
            nc.sync.dma_start(out=X2[64:64 + N, BT:C0], in_=x2_d[N:2 * N, BT:C0])
            nc.sync.dma_start(out=WB[:, o1:o2], in_=wb_d[:, o1:o2])
            nc.sync.dma_start(out=WB[:, o2:o3], in_=wb_d[:, o2:o3])
            nc.sync.dma_start(out=WB[:, o3:F_TOTAL], in_=wb_d[:, o3:F_TOTAL])
            nc.sync.dma_start(out=X2[0:N, C0:H], in_=x2_d[0:N, C0:H])
            nc.sync.dma_start(out=X2[64:64 + N, C0:H], in_=x2_d[N:2 * N, C0:H])

            b_sb = None
            if use_bias:
                b_sb = []
                for i in range(5):
                    nm = len(MCH[i]) if i < 4 else 1
                    t = const.tile([128, nm], f32, tag=f"b{i}")
                    nc.sync.dma_start(out=t, in_=b_d[i].rearrange("m p -> p m"))
                    b_sb.append(t)
            if use_scale:
                asc_sb = const.tile([128, 1], f32, tag="asc")
                abi_sb = const.tile([128, 1], f32, tag="abi")
                nc.sync.dma_start(out=asc_sb, in_=s_d[0].rearrange("m p -> p m"))
                nc.sync.dma_start(out=abi_sb, in_=s_d[1].rearrange("m p -> p m"))

            def w_ap(layer, ki, ms, mc, rb):
                off, kc, dup = WOFF[(layer, ki)]
                base = rb if dup else 0
                return WB[base:base + kc, off + ms:off + ms + mc]

            # eviction engine round-robin: ScalarE (fused act) / VectorE
            rr = [0]

            def evict(layer, mi, h_ap, ps_ap, force=None):
                eng = force
                if eng is None:
                    eng = "s" if rr[0] % 2 == 0 else "v"
                    rr[0] += 1
                if eng == "s":
                    bias = 0.0
                    if use_bias:
                        ob = h_ap.base_partition()
                        mc = h_ap.partition_size()
                        bias = b_sb[layer][ob:ob + mc, mi:mi + 1]
                    nc.scalar.activation(out=h_ap, in_=ps_ap, func=Relu,
                                         bias=bias)
                elif use_bias:
                    ob = h_ap.base_partition()
                    mc = h_ap.partition_size()
                    nc.vector.tensor_scalar(
                        out=h_ap, in0=ps_ap,
                        scalar1=b_sb[layer][ob:ob + mc, mi:mi + 1], scalar2=0.0,
                        op0=mybir.AluOpType.add, op1=mybir.AluOpType.max)
                else:
                    nc.vector.tensor_scalar_max(h_ap, ps_ap, 0.0)

            def s_interleaved(pat):
                zs = [s for s in range(SUB) if pat[s] == 0]
                os_ = [s for s in range(SUB) if pat[s] == 64]
                out = []
                for a, b in zip(zs, os_):
                    out += [a, b]
                return out or list(range(SUB))

            for st in range(N_ST):
                # layer inputs: cur[ki][s] = (tile, row_base, col_off)
                cur = [[(X2, P_PAR[s], (2 * st + s // 2) * BT)
                        for s in range(SUB)]]

                h4 = []   # L4 output chunks for L5
                for layer in range(4):
                    kch = KCH[layer]
                    narrow = [ki for ki, (kc, _p) in enumerate(kch) if kc <= 64]
                    fulls = [ki for ki, (kc, _p) in enumerate(kch) if kc > 64]
                    nxt = []
                    for mi, (ms, mc, opat) in enumerate(MCH[layer]):
                        # merge narrow-k runs across m-chunk pairs
                        order = (narrow + fulls) if mi % 2 == 1 else (fulls + narrow)
                        pss = [psum.tile([128, BT], f32, name="ps", tag="ps")
                               for _ in range(SUB)]
                        for idx, ki in enumerate(order):
                            kc, ipat = kch[ki]
                            pat = ipat if kc <= 64 else (
                                opat if mc <= 64 else P_Z)
                            sseq = (s_interleaved(pat)
                                    if (kc <= 64 or mc <= 64)
                                    else range(SUB))
                            for s in sseq:
                                t_in, ib, co = cur[ki][s]
                                ob = opat[s]
                                nc.tensor.matmul(
                                    pss[s][ob:ob + mc, :],
                                    w_ap(layer, ki, ms, mc, ib),
                                    t_in[ib:ib + kc, co:co + BT],
                                    start=(idx == 0),
                                    stop=(idx == len(order) - 1),
                                    tile_position=(ib, ob))
                        hs = []
                        for s in range(SUB):
                            ob = opat[s]
                            h = acts.tile([128, BT], f16, name="h", tag="h")
                            force = "v" if (layer == 3 and mi == 3) else None
                            evict(layer, mi, h[ob:ob + mc, :],
                                  pss[s][ob:ob + mc, :], force)
                            hs.append((h, ob, 0))
                        nxt.append(hs)
                    cur = nxt
                h4 = cur

                # L5: 512 -> 3 (padded to 32 cols), 4 sub-tiles at column
                # groups 0/32/64/96 of one PSUM bank.
                ps5 = psum.tile([128, BT], f32, name="ps5", tag="ps")
                for ki in range(4):
                    off5 = WOFF[(4, ki)][0]
                    for s in range(SUB):
                        h, ob, _ = h4[ki][s]
                        nc.tensor.matmul(
                            ps5[32 * s:32 * s + 32, :],
                            WB[0:128, off5:off5 + 32],
                            h[0:128, :],
                            start=(ki == 0), stop=(ki == 3),
                            tile_position=(0, 32 * s))
                t5 = outs.tile([128, BT], f32, tag="t5")
                bias5 = b_sb[4][:, 0:1] if use_bias else 0.0
                nc.scalar.activation(out=t5, in_=ps5, func=Tanh, bias=bias5)
                if use_scale:
                    t5s = outs.tile([128, BT], f32, tag="t5s")
                    nc.vector.tensor_scalar(
                        out=t5s, in0=t5, scalar1=asc_sb, scalar2=abi_sb,
                        op0=mybir.AluOpType.mult, op1=mybir.AluOpType.add)
                    t5 = t5s
                for s in range(SUB):
                    T = 4 * st + s
                    nc.sync.dma_start(
                        out=out_d[:, T * BT:(T + 1) * BT],
                        in_=t5[32 * s:32 * s + 3, :])

    nc.compile()
    return nc


_CACHE = {}


def _get_nc(use_bias, use_scale):
    key = (use_bias, use_scale)
    if key not in _CACHE:
        _CACHE[key] = build_nc(use_bias, use_scale)
    return _CACHE[key]


def _pad_bias(vec, layer):
    """Pad a bias vector into the [n_mchunks, 128] layout (dup narrow)."""
    if layer == 4:
        out = np.zeros((1, 128), np.float32)
        for s in range(4):
            out[0, 32 * s:32 * s + 3] = vec[0:3]
        return out
    mch = MCH[layer]
    out = np.zeros((len(mch), 128), np.float32)
    for mi, (ms, mc, _p) in enumerate(mch):
        out[mi, 0:mc] = vec[ms:ms + mc]
        if mc <= 64:
            out[mi, 64:64 + mc] = vec[ms:ms + mc]
    return out


def prepare(inputs):
    """Fold weights and build per-core input maps. Returns (nc, in_maps)."""
    fw = fold_weights(inputs)
    use_bias = any(np.any(bi != 0.0) for bi in fw['b'])
    use_scale = bool(np.any(fw['asc'] != 1.0) or np.any(fw['abi'] != 0.0))
    nc = _get_nc(use_bias, use_scale)

    WB = build_blob(fw)
    x = np.asarray(inputs['x'], np.float32).reshape(B, N)

    in_maps = []
    for c in range(N_CORES):
        m = {"x2": build_x2(x[c * B_CORE:(c + 1) * B_CORE]), "wb": WB}
        if use_bias:
            for i in range(5):
                m[f"bp{i}"] = _pad_bias(fw['b'][i].astype(np.float32), i)
        if use_scale:
            m["ascp"] = _pad_bias(fw['asc'].astype(np.float32), 4)[0:1]
            m["abip"] = _pad_bias(fw['abi'].astype(np.float32), 4)[0:1]
        in_maps.append(m)
    return nc, in_maps


def gather(results):
    """Unshard: per-core out is [3, B_CORE] in natural batch order."""
    parts = [np.asarray(results[c]["out"]).T for c in range(N_CORES)]
    return np.ascontiguousarray(np.concatenate(parts, axis=0)
                                .astype(np.float32))


def kernel(**inputs):
    from concourse.bass_utils import run_bass_kernel_spmd

    nc, in_maps = prepare(inputs)
    res = run_bass_kernel_spmd(nc, in_maps, core_ids=list(range(N_CORES)))
    return gather(res.results)
